# revision 36
# baseline (speedup 1.0000x reference)
"""Trainium2 Bass kernel for nn_ModelIAS_53618371724066 (segment_reduce).

Computes, for each batch row b:
    logits = hidden[b, 1:, :] @ W + b_vec          # [T, S]
    merged[w, :] = mean over {t : seg[b,t] == w} of logits[t, :]   (0 if empty)
    out[b] = merged.T                               # [S, T]

Strategy (data-parallel over batch, 32 rows per core on 8 cores):
  - hidden is quantized host-side to fp8 e3m4 with a per-token scale s_t
    (absmax -> 14.0) so input DMA traffic halves vs fp16; W stays fp16
    (its small magnitudes fall into the e3m4 denormal range).  The matmul
    runs mixed fp8 x fp16 at the bf16 stream rate with fp8 FWL weight
    loads.  Measured end-to-end max rel err ~1.4e-2 vs the 2e-2 gate.
  - The scatter matrix Mg[t, w] = (seg[t] == w) * g[t]/s_t is built in
    ONE DVE tensor_scalar per t-chunk using the two-op form
    (is_equal then mult) -- folding the mean weight g and the fp8
    de-scale into Mg.  This makes the PSUM->SBUF logits copy a pure
    cast, done as a single ACT instruction per row over [128, 2, S].
  - Stage 1 (PE): logits[t_chunk, s] = sum_k hidT[k-chunk].T @ W[k-chunk]
    accumulated in one fp32 PSUM bank per row ([128, 2, S]).
  - Stage 2 (PE): out[s, w] = sum_c lsb_c.T @ Mg_c.  s-channels 0:128 go
    to po1 [128, T]; the 130-wide remainder (2 channels) accumulates into
    a shared [2, 2, T] bank per row-pair so its PSUM->SBUF copy and DMA
    amortize over 2 rows.
  - Outputs leave the chip in fp16 (host converts to f32): po1 is cast by
    DVE into a [128, 8, T] staging tile (one DMA per 8 rows, the last
    group split small so the end-of-program barrier isn't held by a big
    transfer), po2 by ACT into a [128, 4, T] tile.
  - A ~3.5us burst of 8 wide dummy matmuls at program start warms the PE
    HAM clock gate (1.2 -> 2.4 GHz); it is sized to end right as the
    first hidden chunk lands so the PE busy-window never breaks (any
    idle gap resets the HAM un-throttle progress).
  - DMA plumbing: one SP-ring queue carries, in strict FIFO priority,
    a small 2-row first hidden chunk, W, then the remaining hidden
    (whole input fits SBUF) and all output DMAs; the tiny segt/gst
    constants ride the otherwise-idle GPSIMD ring.  DMA trigger
    instructions cost ~0.6us on the issuing engine and block on ring
    capacity, so none may live on ACT/DVE which do per-row work.
  - A dummy ACTIVATE after program start pulls the lazy 1.3us ACT table
    load into the startup window instead of row 0's critical path.
  - Per-instruction sem-waits are legalized for the pinned walrus by
    _split_sync_waits.

Measured 53.4us (from 73.9us baseline): preamble ~8 | warmup ~3.5 |
32 rows x 1134ns | tail+teardown ~4.  The row period is the PE floor:
stage1 streams at the 54ns/matmul fp16 rate and stage2 at 108ns/matmul
(and the DVE is co-bound at ~1095ns/row of Mg-build + output-cast work).
"""

import numpy as np
import ml_dtypes

import concourse.bass as bass
import concourse.tile as tile
from concourse import mybir
from concourse.bass_utils import run_bass_kernel_spmd

B, T, H, S = 256, 256, 768, 130
N_CORES = 8
RPC = B // N_CORES  # rows per core
KCH = H // 128  # k chunks of the hidden dim
F32 = mybir.dt.float32
HP = mybir.dt.float16
FP8 = mybir.dt.float8e3  # e3m4
N_WARM = 7
ROWS_PER_HDMA = 4
HDMA_GROUPS = [2, 2, 4, 4, 4, 4, 4, 4, 4]


def _split_sync_waits(nc):
    """The pinned walrus build rejects instructions carrying more than one
    sync-wait command ("Too many sync wait commands", setupSyncWait).  Keep
    one wait per instruction and hoist the rest onto NoOps inserted just
    before it on the same engine (same semantics: all waits still execute
    before the instruction, in stream order)."""
    for f in nc.m.functions:
        for blk in f.blocks:
            il = blk.instructions
            i = 0
            while i < len(il):
                inst = il[i]
                si = inst.sync_info
                if si is not None and si.on_wait and len(si.on_wait) >= 2:
                    waits = list(si.on_wait)
                    keep = [waits.pop()]
                    pos = i
                    for j, w in enumerate(waits):
                        nop = mybir.InstNoOp(name=f"{inst.name}_ws{j}", ins=[], outs=[])
                        nop.engine = inst.engine
                        nop.sync_info = mybir.SyncInfo(on_wait=[w], on_update=[])
                        il.insert(pos, nop)
                        pos += 1
                        i += 1
                    inst.sync_info = mybir.SyncInfo(
                        on_wait=keep, on_update=list(si.on_update)
                    )
                i += 1


def _build_program(rpc=RPC, with_bias=False, split_waits=True):
    nc = bass.Bass("TRN2", target_bir_lowering=False, debug=False)

    hid = nc.dram_tensor("hiddent", [128, rpc, KCH, T], FP8, kind="ExternalInput")
    w_d = nc.dram_tensor("w", [128, KCH, S], HP, kind="ExternalInput")
    b_d = nc.dram_tensor("bvec", [1, S], HP, kind="ExternalInput")
    seg_d = nc.dram_tensor("segt", [128, 2, rpc], F32, kind="ExternalInput")
    g_d = nc.dram_tensor("gst", [128, 2, rpc], F32, kind="ExternalInput")
    out_d = nc.dram_tensor("out", [128, rpc, 2, S], HP, kind="ExternalOutput")

    eq = mybir.AluOpType.is_equal
    mul = mybir.AluOpType.mult
    copyf = mybir.ActivationFunctionType.Copy
    assert rpc % ROWS_PER_HDMA == 0
    ngrp = rpc // ROWS_PER_HDMA
    with tile.TileContext(nc) as tc:
        with (
            tc.tile_pool(name="const", bufs=1) as const_pool,
            tc.tile_pool(name="hid", bufs=len(HDMA_GROUPS)) as hid_pool,
            tc.tile_pool(name="mgp", bufs=3) as mg_pool,
            tc.tile_pool(name="lsbp", bufs=3) as lsb_pool,
            tc.tile_pool(name="ob1p", bufs=3) as ob1_pool,
            tc.tile_pool(name="warm", bufs=1, space=bass.MemorySpace.PSUM) as warm_pool,
            tc.tile_pool(name="psl", bufs=2, space=bass.MemorySpace.PSUM) as psl_pool,
            tc.tile_pool(name="po1", bufs=2, space=bass.MemorySpace.PSUM) as po1_pool,
        ):
            # --- HAM warmup: keep the PE busy from t~0 so the clock gate
            # opens (K=8/8) before real matmuls arrive.  Dummy data.
            wz = const_pool.tile([128, 512], HP)
            nc.vector.memset(wz[:], 0.0)
            pwu = warm_pool.tile([64, 512], F32)
            for i in range(N_WARM):
                nc.tensor.matmul(
                    pwu[:, :],
                    wz[:, 0:64],
                    wz[:, :],
                    start=True,
                    stop=True,
                    skip_group_check=True,
                )

            # --- all input DMAs ride ONE SP-ring queue in priority order:
            # tiny constants first (so per-row deps clear by ~9us), then the
            # whole hidden tensor.  The first two hidden chunks are 2 rows
            # each so row 0 can start as early as possible.  The SP engine
            # has no per-row duties, so trigger instructions blocking on
            # DMA-ring capacity cost nothing.
            hts = []  # (row_start, n_rows, tile)
            ht0 = hid_pool.tile([128, HDMA_GROUPS[0], KCH, T], FP8, tag="ht0", name="htt")
            nc.sync.dma_start(ht0[:], hid.ap()[:, 0 : HDMA_GROUPS[0]])
            hts.append((0, HDMA_GROUPS[0], ht0))
            wt = const_pool.tile([128, KCH, S], HP)
            nc.sync.dma_start(wt[:], w_d.ap()[:])
            # tiny per-row constants ride the otherwise-idle GPSIMD ring so
            # they don't queue behind hidden chunks
            segt = const_pool.tile([128, 2, rpc], F32)
            nc.gpsimd.dma_start(segt[:], seg_d.ap()[:])
            gst = const_pool.tile([128, 2, rpc], F32)
            nc.gpsimd.dma_start(gst[:], g_d.ap()[:])
            row0 = HDMA_GROUPS[0]
            for nrow in HDMA_GROUPS[1:]:
                ht = hid_pool.tile([128, nrow, KCH, T], FP8, tag=f"ht{nrow}", name="htt")
                nc.sync.dma_start(ht[:], hid.ap()[:, row0 : row0 + nrow])
                hts.append((row0, nrow, ht))
                row0 += nrow
            assert row0 == rpc

            def hid_slice(r):
                for row0, nrow, ht in hts:
                    if row0 <= r < row0 + nrow:
                        return ht[:, r - row0]
                raise AssertionError

            # dummy ACTIVATE pulls the lazy ACT table load into the idle
            # startup window instead of the first row's critical path
            wz2 = const_pool.tile([1, 8], HP)
            nc.scalar.copy(wz2[:], wz[0:1, 0:8])
            iota_i = const_pool.tile([128, T], mybir.dt.int32)
            nc.gpsimd.iota(iota_i[:], pattern=[[1, T]], base=0, channel_multiplier=0)
            iota16 = const_pool.tile([128, T], HP)
            nc.vector.tensor_copy(iota16[:], iota_i[:])
            if with_bias:
                ones = const_pool.tile([1, 128], HP)
                nc.vector.memset(ones[:], 1.0)
                bsb = const_pool.tile([1, S], HP)
                nc.scalar.dma_start(bsb[:], b_d.ap()[:])

            pending = []
            ob1s = {}

            def emit_stage2(item):
                # out[w, s] = sum_t Mg[t, w] * lsb[t, s]: Mg is the stationary
                # (two full M=128 w-chunks -- no 130-wide remainder group) and
                # lsb streams at N=130, halving stage-2 stream cycles vs the
                # lsb-stationary form.  Host transposes [w, s] -> [s, t].
                pr, plsb, pmg = item
                g8, rr8 = divmod(pr, 8)
                po = po1_pool.tile([128, 2, S], F32, tag="po")
                for wc in range(2):
                    for tch in range(2):
                        nc.tensor.matmul(
                            po[:, wc, :],
                            pmg[:, tch, 128 * wc : 128 * (wc + 1)],
                            plsb[:, tch, :],
                            start=(tch == 0),
                            stop=(tch == 1),
                            skip_group_check=True,
                        )
                if rr8 == 0:
                    ob1s[g8] = ob1_pool.tile([128, 8, 2, S], HP, tag="ob1", name="ob1t")
                ob1 = ob1s[g8]
                nc.vector.tensor_copy(ob1[:, rr8, :, :], po[:])
                if g8 == 3 and rr8 == 3:
                    # split the last group's DMAs so the final transfer is small
                    nc.sync.dma_start(out_d.ap()[:, 24:28], ob1[:, 0:4])
                if g8 == 3 and rr8 == 5:
                    nc.sync.dma_start(out_d.ap()[:, 28:30], ob1[:, 4:6])
                if rr8 == 7:
                    if g8 == 3:
                        nc.sync.dma_start(out_d.ap()[:, 30:32], ob1[:, 6:8])
                    else:
                        nc.sync.dma_start(
                            out_d.ap()[:, 8 * g8 : 8 * (g8 + 1)], ob1[:]
                        )
                    del ob1s[g8]

            for r in range(rpc):
                ht_r = hid_slice(r)

                # Mg[t, w] = (seg[t] == w) * g[t]/s_t in fp16, one DVE
                # two-op tensor_scalar per t-chunk
                mg = mg_pool.tile([128, 2, T], HP, tag="mg")
                for c, e in ((0, nc.vector), (1, nc.gpsimd)):
                    e.tensor_scalar(
                        mg[:, c, :],
                        iota16[:],
                        segt[:, c, r : r + 1],
                        gst[:, c, r : r + 1],
                        eq,
                        mul,
                    )

                # stage 1: scaled logits for both t-chunks into one PSUM bank
                psl = psl_pool.tile([128, 2, S], F32, tag="psl")
                for c in range(2):
                    for k in range(KCH):
                        nc.tensor.matmul(
                            psl[:, c, :],
                            ht_r[:, k, 128 * c : 128 * (c + 1)],
                            wt[:, k, :],
                            start=(k == 0),
                            stop=(k == KCH - 1 and not with_bias),
                            skip_group_check=True,
                        )
                    if with_bias:
                        nc.tensor.matmul(
                            psl[:, c, :], ones[:], bsb[:], start=False, stop=True,
                            skip_group_check=True,
                        )

                # PSUM -> SBUF pure cast, one ACT instruction per row
                lsb = lsb_pool.tile([128, 2, S], HP, tag="lsb")
                nc.scalar.activation(lsb[:], psl[:], copyf)

                # stage 2 is emitted one row late (software pipeline) so the
                # PE never waits on the ACT/DVE work of the same row.
                pending.append((r, lsb, mg))
                if len(pending) > 1:
                    emit_stage2(pending.pop(0))
            while pending:
                emit_stage2(pending.pop(0))

    if split_waits:
        _split_sync_waits(nc)
    return nc


def _host_prep(hidden, W, b, seg):
    """Layout/encoding prep: fp8 e3m4 per-token quantization of hidden,
    1/count of the integer segment ids, partition-major packing."""
    h = np.ascontiguousarray(hidden[:, 1:, :], dtype=np.float32)
    absmax = np.abs(h).max(axis=2, keepdims=True)  # [B, T, 1]
    s_t = (14.0 / np.maximum(absmax, 1e-30)).astype(np.float32)
    h8 = (h * s_t).astype(ml_dtypes.float8_e3m4)
    # [core][p, r, k, t] with p the SBUF partition (= h % 128 within chunk k)
    h8 = h8.reshape(N_CORES, RPC, T, KCH, 128)
    hiddenT = np.ascontiguousarray(h8.transpose(0, 4, 1, 3, 2))

    seg = np.asarray(seg)
    counts = np.zeros((B, T), dtype=np.int64)
    rows = np.arange(B)[:, None]
    np.add.at(counts, (rows, seg), 1)
    g = (1.0 / np.maximum(counts, 1))[rows, seg].astype(np.float32)  # [B, T]
    gs = (g / s_t[:, :, 0]).astype(np.float32)
    segf = seg.astype(np.float32)

    # partition-major packing: [core][p, c, r] = value at (row0+r, 128c+p)
    def pack(x):
        x4 = x.reshape(N_CORES, RPC, 2, 128)  # [core, r, c, p]
        return np.ascontiguousarray(x4.transpose(0, 3, 2, 1))

    segt = pack(segf)
    gst = pack(gs)
    w16 = np.asarray(W, dtype=np.float32).astype(np.float16).reshape(KCH, 128, S)
    w_in = np.ascontiguousarray(w16.transpose(1, 0, 2))  # [128, KCH, S]
    b_in = np.ascontiguousarray(b, dtype=np.float32).astype(np.float16).reshape(1, S)
    return hiddenT, w_in, b_in, segt, gst


_CACHE = {}


def kernel(hidden, W, b, seg):
    hiddenT, w_in, b_in, segt, gst = _host_prep(hidden, W, b, seg)
    with_bias = bool(np.any(b_in != 0.0))

    key = ("prog", with_bias)
    if key not in _CACHE:
        _CACHE[key] = _build_program(with_bias=with_bias)
    nc = _CACHE[key]

    in_maps = []
    for c in range(N_CORES):
        in_maps.append(
            {
                "hiddent": hiddenT[c],
                "w": w_in,
                "bvec": b_in,
                "segt": segt[c],
                "gst": gst[c],
            }
        )
    res = run_bass_kernel_spmd(nc, in_maps, core_ids=list(range(N_CORES)))
    # device layout: out [128(w%128), RPC, 2(w//128), S]; t = 128*wc + p
    out = np.empty((B, S, T), dtype=np.float32)
    for c in range(N_CORES):
        o = np.asarray(res.results[c]["out"], dtype=np.float32)
        out[c * RPC : (c + 1) * RPC] = (
            o.transpose(1, 3, 2, 0).reshape(RPC, S, T)
        )
    return out


# revision 37
# speedup vs baseline: 2.6724x; 2.6724x over previous
"""Trainium2 Bass kernel for nn_ModelIAS_53618371724066 (segment_reduce).

Computes, for each batch row b:
    logits = hidden[b, 1:, :] @ W + b_vec          # [T, S]
    merged[w, :] = mean over {t : seg[b,t] == w} of logits[t, :]   (0 if empty)
    out[b] = merged.T                               # [S, T]

Strategy (data-parallel over batch, 32 rows per core on 8 cores):
  - hidden is quantized host-side to fp8 e3m4 with a per-token scale s_t
    (absmax -> 14.0) so input DMA traffic halves vs fp16; W stays fp16
    (its small magnitudes fall into the e3m4 denormal range).  The matmul
    runs mixed fp8 x fp16 at the bf16 stream rate with fp8 FWL weight
    loads.  Measured end-to-end max rel err ~1.4e-2 vs the 2e-2 gate.
  - The scatter matrix Mg[t, w] = (seg[t] == w) * g[t]/s_t is built in
    ONE DVE tensor_scalar per t-chunk using the two-op form
    (is_equal then mult) -- folding the mean weight g and the fp8
    de-scale into Mg.  This makes the PSUM->SBUF logits copy a pure
    cast, done as a single ACT instruction per row over [128, 2, S].
  - Stage 1 (PE): logits[t_chunk, s] = sum_k hidT[k-chunk].T @ W[k-chunk]
    accumulated in one fp32 PSUM bank per row ([128, 2, S]).
  - Stage 2 (PE): out[s, w] = sum_c lsb_c.T @ Mg_c.  s-channels 0:128 go
    to po1 [128, T]; the 130-wide remainder (2 channels) accumulates into
    a shared [2, 2, T] bank per row-pair so its PSUM->SBUF copy and DMA
    amortize over 2 rows.
  - Outputs leave the chip in fp16 (host converts to f32): po1 is cast by
    DVE into a [128, 8, T] staging tile (one DMA per 8 rows, the last
    group split small so the end-of-program barrier isn't held by a big
    transfer), po2 by ACT into a [128, 4, T] tile.
  - A ~3.5us burst of 8 wide dummy matmuls at program start warms the PE
    HAM clock gate (1.2 -> 2.4 GHz); it is sized to end right as the
    first hidden chunk lands so the PE busy-window never breaks (any
    idle gap resets the HAM un-throttle progress).
  - DMA plumbing: one SP-ring queue carries, in strict FIFO priority,
    a small 2-row first hidden chunk, W, then the remaining hidden
    (whole input fits SBUF) and all output DMAs; the tiny segt/gst
    constants ride the otherwise-idle GPSIMD ring.  DMA trigger
    instructions cost ~0.6us on the issuing engine and block on ring
    capacity, so none may live on ACT/DVE which do per-row work.
  - A dummy ACTIVATE after program start pulls the lazy 1.3us ACT table
    load into the startup window instead of row 0's critical path.
  - Per-instruction sem-waits are legalized for the pinned walrus by
    _split_sync_waits.

Measured 53.4us (from 73.9us baseline): preamble ~8 | warmup ~3.5 |
32 rows x 1134ns | tail+teardown ~4.  The row period is the PE floor:
stage1 streams at the 54ns/matmul fp16 rate and stage2 at 108ns/matmul
(and the DVE is co-bound at ~1095ns/row of Mg-build + output-cast work).
"""

import numpy as np
import ml_dtypes

import concourse.bass as bass
import concourse.tile as tile
from concourse import mybir
from concourse.bass_utils import run_bass_kernel_spmd

B, T, H, S = 256, 256, 768, 130
N_CORES = 8
RPC = B // N_CORES  # rows per core
KCH = H // 128  # k chunks of the hidden dim
F32 = mybir.dt.float32
HP = mybir.dt.float16
FP8 = mybir.dt.float8e3  # e3m4
N_WARM = 7
ROWS_PER_HDMA = 4
HDMA_GROUPS = [2, 2, 4, 4, 4, 4, 4, 4, 4]


def _split_sync_waits(nc):
    """The pinned walrus build rejects instructions carrying more than one
    sync-wait command ("Too many sync wait commands", setupSyncWait).  Keep
    one wait per instruction and hoist the rest onto NoOps inserted just
    before it on the same engine (same semantics: all waits still execute
    before the instruction, in stream order)."""
    for f in nc.m.functions:
        for blk in f.blocks:
            il = blk.instructions
            i = 0
            while i < len(il):
                inst = il[i]
                si = inst.sync_info
                if si is not None and si.on_wait and len(si.on_wait) >= 2:
                    waits = list(si.on_wait)
                    keep = [waits.pop()]
                    pos = i
                    for j, w in enumerate(waits):
                        nop = mybir.InstNoOp(name=f"{inst.name}_ws{j}", ins=[], outs=[])
                        nop.engine = inst.engine
                        nop.sync_info = mybir.SyncInfo(on_wait=[w], on_update=[])
                        il.insert(pos, nop)
                        pos += 1
                        i += 1
                    inst.sync_info = mybir.SyncInfo(
                        on_wait=keep, on_update=list(si.on_update)
                    )
                i += 1


def _build_program(rpc=RPC, with_bias=False, split_waits=True):
    nc = bass.Bass("TRN2", target_bir_lowering=False, debug=False)

    hid = nc.dram_tensor("hiddent", [128, rpc, KCH, T], FP8, kind="ExternalInput")
    w_d = nc.dram_tensor("w", [128, KCH, S], HP, kind="ExternalInput")
    b_d = nc.dram_tensor("bvec", [1, S], HP, kind="ExternalInput")
    seg_d = nc.dram_tensor("segt", [128, 2, rpc], F32, kind="ExternalInput")
    g_d = nc.dram_tensor("gst", [128, 2, rpc], F32, kind="ExternalInput")
    out_d = nc.dram_tensor("out", [128, rpc, 2, S], HP, kind="ExternalOutput")

    eq = mybir.AluOpType.is_equal
    mul = mybir.AluOpType.mult
    copyf = mybir.ActivationFunctionType.Copy
    assert rpc % ROWS_PER_HDMA == 0
    ngrp = rpc // ROWS_PER_HDMA
    with tile.TileContext(nc) as tc:
        with (
            tc.tile_pool(name="const", bufs=1) as const_pool,
            tc.tile_pool(name="hid", bufs=len(HDMA_GROUPS)) as hid_pool,
            tc.tile_pool(name="mgp", bufs=3) as mg_pool,
            tc.tile_pool(name="lsbp", bufs=3) as lsb_pool,
            tc.tile_pool(name="ob1p", bufs=3) as ob1_pool,
            tc.tile_pool(name="warm", bufs=1, space=bass.MemorySpace.PSUM) as warm_pool,
            tc.tile_pool(name="psl", bufs=2, space=bass.MemorySpace.PSUM) as psl_pool,
            tc.tile_pool(name="po1", bufs=2, space=bass.MemorySpace.PSUM) as po1_pool,
        ):
            # --- HAM warmup: keep the PE busy from t~0 so the clock gate
            # opens (K=8/8) before real matmuls arrive.  Dummy data.
            wz = const_pool.tile([128, 512], HP)
            nc.vector.memset(wz[:], 0.0)
            pwu = warm_pool.tile([64, 512], F32)
            for i in range(N_WARM):
                nc.tensor.matmul(
                    pwu[:, :],
                    wz[:, 0:64],
                    wz[:, :],
                    start=True,
                    stop=True,
                    skip_group_check=True,
                )

            # --- all input DMAs ride ONE SP-ring queue in priority order:
            # tiny constants first (so per-row deps clear by ~9us), then the
            # whole hidden tensor.  The first two hidden chunks are 2 rows
            # each so row 0 can start as early as possible.  The SP engine
            # has no per-row duties, so trigger instructions blocking on
            # DMA-ring capacity cost nothing.
            hts = []  # (row_start, n_rows, tile)
            ht0 = hid_pool.tile([128, HDMA_GROUPS[0], KCH, T], FP8, tag="ht0", name="htt")
            nc.sync.dma_start(ht0[:], hid.ap()[:, 0 : HDMA_GROUPS[0]])
            hts.append((0, HDMA_GROUPS[0], ht0))
            wt = const_pool.tile([128, KCH, S], HP)
            nc.sync.dma_start(wt[:], w_d.ap()[:])
            # tiny per-row constants ride the otherwise-idle GPSIMD ring so
            # they don't queue behind hidden chunks
            segt = const_pool.tile([128, 2, rpc], F32)
            nc.gpsimd.dma_start(segt[:], seg_d.ap()[:])
            gst = const_pool.tile([128, 2, rpc], F32)
            nc.gpsimd.dma_start(gst[:], g_d.ap()[:])
            row0 = HDMA_GROUPS[0]
            for nrow in HDMA_GROUPS[1:]:
                ht = hid_pool.tile([128, nrow, KCH, T], FP8, tag=f"ht{nrow}", name="htt")
                nc.sync.dma_start(ht[:], hid.ap()[:, row0 : row0 + nrow])
                hts.append((row0, nrow, ht))
                row0 += nrow
            assert row0 == rpc

            def hid_slice(r):
                for row0, nrow, ht in hts:
                    if row0 <= r < row0 + nrow:
                        return ht[:, r - row0]
                raise AssertionError

            # dummy ACTIVATE pulls the lazy ACT table load into the idle
            # startup window instead of the first row's critical path
            wz2 = const_pool.tile([1, 8], HP)
            nc.scalar.copy(wz2[:], wz[0:1, 0:8])
            iota_i = const_pool.tile([128, T], mybir.dt.int32)
            nc.gpsimd.iota(iota_i[:], pattern=[[1, T]], base=0, channel_multiplier=0)
            iota16 = const_pool.tile([128, T], HP)
            nc.vector.tensor_copy(iota16[:], iota_i[:])
            if with_bias:
                ones = const_pool.tile([1, 128], HP)
                nc.vector.memset(ones[:], 1.0)
                bsb = const_pool.tile([1, S], HP)
                nc.scalar.dma_start(bsb[:], b_d.ap()[:])

            pending = []
            ob1s = {}

            def emit_stage2(item):
                # out[w, s] = sum_t Mg[t, w] * lsb[t, s]: Mg is the stationary
                # (two full M=128 w-chunks -- no 130-wide remainder group) and
                # lsb streams at N=130, halving stage-2 stream cycles vs the
                # lsb-stationary form.  Host transposes [w, s] -> [s, t].
                pr, plsb, pmg = item
                g8, rr8 = divmod(pr, 8)
                po = po1_pool.tile([128, 2, S], F32, tag="po")
                for wc in range(2):
                    for tch in range(2):
                        nc.tensor.matmul(
                            po[:, wc, :],
                            pmg[:, tch, 128 * wc : 128 * (wc + 1)],
                            plsb[:, tch, :],
                            start=(tch == 0),
                            stop=(tch == 1),
                            skip_group_check=True,
                        )
                if rr8 == 0:
                    ob1s[g8] = ob1_pool.tile([128, 8, 2, S], HP, tag="ob1", name="ob1t")
                ob1 = ob1s[g8]
                if pr % 2 == 0:
                    nc.scalar.copy(ob1[:, rr8, :, :], po[:])
                else:
                    nc.vector.tensor_copy(ob1[:, rr8, :, :], po[:])
                if g8 == 3 and rr8 == 3:
                    # split the last group's DMAs so the final transfer is small
                    nc.sync.dma_start(out_d.ap()[:, 24:28], ob1[:, 0:4])
                if g8 == 3 and rr8 == 5:
                    nc.sync.dma_start(out_d.ap()[:, 28:30], ob1[:, 4:6])
                if rr8 == 7:
                    if g8 == 3:
                        nc.sync.dma_start(out_d.ap()[:, 30:32], ob1[:, 6:8])
                    else:
                        nc.sync.dma_start(
                            out_d.ap()[:, 8 * g8 : 8 * (g8 + 1)], ob1[:]
                        )
                    del ob1s[g8]

            for r in range(rpc):
                ht_r = hid_slice(r)

                # Mg[t, w] = (seg[t] == w) * g[t]/s_t in fp16, one DVE
                # two-op tensor_scalar per t-chunk
                mg = mg_pool.tile([128, 2, T], HP, tag="mg")
                for c in range(2):
                    nc.vector.tensor_scalar(
                        mg[:, c, :],
                        iota16[:],
                        segt[:, c, r : r + 1],
                        gst[:, c, r : r + 1],
                        eq,
                        mul,
                    )

                # stage 1: scaled logits for both t-chunks into one PSUM bank
                psl = psl_pool.tile([128, 2, S], F32, tag="psl")
                for c in range(2):
                    for k in range(KCH):
                        nc.tensor.matmul(
                            psl[:, c, :],
                            ht_r[:, k, 128 * c : 128 * (c + 1)],
                            wt[:, k, :],
                            start=(k == 0),
                            stop=(k == KCH - 1 and not with_bias),
                            skip_group_check=True,
                        )
                    if with_bias:
                        nc.tensor.matmul(
                            psl[:, c, :], ones[:], bsb[:], start=False, stop=True,
                            skip_group_check=True,
                        )

                # PSUM -> SBUF pure cast, one ACT instruction per row
                lsb = lsb_pool.tile([128, 2, S], HP, tag="lsb")
                nc.scalar.activation(lsb[:], psl[:], copyf)

                # stage 2 is emitted one row late (software pipeline) so the
                # PE never waits on the ACT/DVE work of the same row.
                pending.append((r, lsb, mg))
                if len(pending) > 1:
                    emit_stage2(pending.pop(0))
            while pending:
                emit_stage2(pending.pop(0))

    if split_waits:
        _split_sync_waits(nc)
    return nc


def _host_prep(hidden, W, b, seg):
    """Layout/encoding prep: fp8 e3m4 per-token quantization of hidden,
    1/count of the integer segment ids, partition-major packing."""
    h = np.ascontiguousarray(hidden[:, 1:, :], dtype=np.float32)
    absmax = np.abs(h).max(axis=2, keepdims=True)  # [B, T, 1]
    s_t = (14.0 / np.maximum(absmax, 1e-30)).astype(np.float32)
    h8 = (h * s_t).astype(ml_dtypes.float8_e3m4)
    # [core][p, r, k, t] with p the SBUF partition (= h % 128 within chunk k)
    h8 = h8.reshape(N_CORES, RPC, T, KCH, 128)
    hiddenT = np.ascontiguousarray(h8.transpose(0, 4, 1, 3, 2))

    seg = np.asarray(seg)
    counts = np.zeros((B, T), dtype=np.int64)
    rows = np.arange(B)[:, None]
    np.add.at(counts, (rows, seg), 1)
    g = (1.0 / np.maximum(counts, 1))[rows, seg].astype(np.float32)  # [B, T]
    gs = (g / s_t[:, :, 0]).astype(np.float32)
    segf = seg.astype(np.float32)

    # partition-major packing: [core][p, c, r] = value at (row0+r, 128c+p)
    def pack(x):
        x4 = x.reshape(N_CORES, RPC, 2, 128)  # [core, r, c, p]
        return np.ascontiguousarray(x4.transpose(0, 3, 2, 1))

    segt = pack(segf)
    gst = pack(gs)
    w16 = np.asarray(W, dtype=np.float32).astype(np.float16).reshape(KCH, 128, S)
    w_in = np.ascontiguousarray(w16.transpose(1, 0, 2))  # [128, KCH, S]
    b_in = np.ascontiguousarray(b, dtype=np.float32).astype(np.float16).reshape(1, S)
    return hiddenT, w_in, b_in, segt, gst


_CACHE = {}


def kernel(hidden, W, b, seg):
    hiddenT, w_in, b_in, segt, gst = _host_prep(hidden, W, b, seg)
    with_bias = bool(np.any(b_in != 0.0))

    key = ("prog", with_bias)
    if key not in _CACHE:
        _CACHE[key] = _build_program(with_bias=with_bias)
    nc = _CACHE[key]

    in_maps = []
    for c in range(N_CORES):
        in_maps.append(
            {
                "hiddent": hiddenT[c],
                "w": w_in,
                "bvec": b_in,
                "segt": segt[c],
                "gst": gst[c],
            }
        )
    res = run_bass_kernel_spmd(nc, in_maps, core_ids=list(range(N_CORES)))
    # device layout: out [128(w%128), RPC, 2(w//128), S]; t = 128*wc + p
    out = np.empty((B, S, T), dtype=np.float32)
    for c in range(N_CORES):
        o = np.asarray(res.results[c]["out"], dtype=np.float32)
        out[c * RPC : (c + 1) * RPC] = (
            o.transpose(1, 3, 2, 0).reshape(RPC, S, T)
        )
    return out


# revision 38
# speedup vs baseline: 3.0841x; 1.1540x over previous
"""Trainium2 Bass kernel for nn_ModelIAS_53618371724066 (segment_reduce).

Computes, for each batch row b:
    logits = hidden[b, 1:, :] @ W + b_vec          # [T, S]
    merged[w, :] = mean over {t : seg[b,t] == w} of logits[t, :]   (0 if empty)
    out[b] = merged.T                               # [S, T]

Strategy (data-parallel over batch, 32 rows per core on 8 cores):
  - hidden is quantized host-side to fp8 e3m4 with a per-token scale s_t
    (absmax -> 14.0) so input DMA traffic halves vs fp16; W stays fp16
    (its small magnitudes fall into the e3m4 denormal range).  The matmul
    runs mixed fp8 x fp16 at the bf16 stream rate with fp8 FWL weight
    loads.  Measured end-to-end max rel err ~1.4e-2 vs the 2e-2 gate.
  - The scatter matrix Mg[t, w] = (seg[t] == w) * g[t]/s_t is built in
    ONE DVE tensor_scalar per t-chunk using the two-op form
    (is_equal then mult) -- folding the mean weight g and the fp8
    de-scale into Mg.  This makes the PSUM->SBUF logits copy a pure
    cast, done as a single ACT instruction per row over [128, 2, S].
  - Stage 1 (PE): logits[t_chunk, s] = sum_k hidT[k-chunk].T @ W[k-chunk]
    accumulated in one fp32 PSUM bank per row ([128, 2, S]).
  - Stage 2 (PE): out[s, w] = sum_c lsb_c.T @ Mg_c.  s-channels 0:128 go
    to po1 [128, T]; the 130-wide remainder (2 channels) accumulates into
    a shared [2, 2, T] bank per row-pair so its PSUM->SBUF copy and DMA
    amortize over 2 rows.
  - Outputs leave the chip in fp16 (host converts to f32): po1 is cast by
    DVE into a [128, 8, T] staging tile (one DMA per 8 rows, the last
    group split small so the end-of-program barrier isn't held by a big
    transfer), po2 by ACT into a [128, 4, T] tile.
  - A ~3.5us burst of 8 wide dummy matmuls at program start warms the PE
    HAM clock gate (1.2 -> 2.4 GHz); it is sized to end right as the
    first hidden chunk lands so the PE busy-window never breaks (any
    idle gap resets the HAM un-throttle progress).
  - DMA plumbing: one SP-ring queue carries, in strict FIFO priority,
    a small 2-row first hidden chunk, W, then the remaining hidden
    (whole input fits SBUF) and all output DMAs; the tiny segt/gst
    constants ride the otherwise-idle GPSIMD ring.  DMA trigger
    instructions cost ~0.6us on the issuing engine and block on ring
    capacity, so none may live on ACT/DVE which do per-row work.
  - A dummy ACTIVATE after program start pulls the lazy 1.3us ACT table
    load into the startup window instead of row 0's critical path.
  - Per-instruction sem-waits are legalized for the pinned walrus by
    _split_sync_waits.

Measured 53.4us (from 73.9us baseline): preamble ~8 | warmup ~3.5 |
32 rows x 1134ns | tail+teardown ~4.  The row period is the PE floor:
stage1 streams at the 54ns/matmul fp16 rate and stage2 at 108ns/matmul
(and the DVE is co-bound at ~1095ns/row of Mg-build + output-cast work).
"""

import numpy as np
import ml_dtypes

import concourse.bass as bass
import concourse.tile as tile
from concourse import mybir
from concourse.bass_utils import run_bass_kernel_spmd

B, T, H, S = 256, 256, 768, 130
N_CORES = 8
RPC = B // N_CORES  # rows per core
KCH = H // 128  # k chunks of the hidden dim
F32 = mybir.dt.float32
HP = mybir.dt.float16
FP8 = mybir.dt.float8e3  # e3m4
N_WARM = 7
ROWS_PER_HDMA = 4
HDMA_GROUPS = [2, 2, 4, 4, 4, 4, 4, 4, 4]


def _split_sync_waits(nc):
    """The pinned walrus build rejects instructions carrying more than one
    sync-wait command ("Too many sync wait commands", setupSyncWait).  Keep
    one wait per instruction and hoist the rest onto NoOps inserted just
    before it on the same engine (same semantics: all waits still execute
    before the instruction, in stream order)."""
    for f in nc.m.functions:
        for blk in f.blocks:
            il = blk.instructions
            i = 0
            while i < len(il):
                inst = il[i]
                si = inst.sync_info
                if si is not None and si.on_wait and len(si.on_wait) >= 2:
                    waits = list(si.on_wait)
                    keep = [waits.pop()]
                    pos = i
                    for j, w in enumerate(waits):
                        nop = mybir.InstNoOp(name=f"{inst.name}_ws{j}", ins=[], outs=[])
                        nop.engine = inst.engine
                        nop.sync_info = mybir.SyncInfo(on_wait=[w], on_update=[])
                        il.insert(pos, nop)
                        pos += 1
                        i += 1
                    inst.sync_info = mybir.SyncInfo(
                        on_wait=keep, on_update=list(si.on_update)
                    )
                i += 1


def _build_program(rpc=RPC, with_bias=False, split_waits=True):
    nc = bass.Bass("TRN2", target_bir_lowering=False, debug=False)

    hid = nc.dram_tensor("hiddent", [128, rpc, KCH, T], FP8, kind="ExternalInput")
    w_d = nc.dram_tensor("w", [128, KCH, S], HP, kind="ExternalInput")
    b_d = nc.dram_tensor("bvec", [1, S], HP, kind="ExternalInput")
    seg_d = nc.dram_tensor("segt", [128, 2, rpc], F32, kind="ExternalInput")
    g_d = nc.dram_tensor("gst", [128, 2, rpc], F32, kind="ExternalInput")
    out_d = nc.dram_tensor("out", [128, rpc, 2, S], HP, kind="ExternalOutput")

    eq = mybir.AluOpType.is_equal
    mul = mybir.AluOpType.mult
    copyf = mybir.ActivationFunctionType.Copy
    assert rpc % ROWS_PER_HDMA == 0
    ngrp = rpc // ROWS_PER_HDMA
    with tile.TileContext(nc) as tc:
        with (
            tc.tile_pool(name="const", bufs=1) as const_pool,
            tc.tile_pool(name="hid", bufs=len(HDMA_GROUPS)) as hid_pool,
            tc.tile_pool(name="mgp", bufs=3) as mg_pool,
            tc.tile_pool(name="lsbp", bufs=3) as lsb_pool,
            tc.tile_pool(name="ob1p", bufs=3) as ob1_pool,
            tc.tile_pool(name="warm", bufs=1, space=bass.MemorySpace.PSUM) as warm_pool,
            tc.tile_pool(name="psl", bufs=2, space=bass.MemorySpace.PSUM) as psl_pool,
            tc.tile_pool(name="po1", bufs=2, space=bass.MemorySpace.PSUM) as po1_pool,
        ):
            # --- HAM warmup: keep the PE busy from t~0 so the clock gate
            # opens (K=8/8) before real matmuls arrive.  Dummy data.
            wz = const_pool.tile([128, 512], HP)
            nc.vector.memset(wz[:], 0.0)
            pwu = warm_pool.tile([64, 512], F32)
            for i in range(N_WARM):
                nc.tensor.matmul(
                    pwu[:, :],
                    wz[:, 0:64],
                    wz[:, :],
                    start=True,
                    stop=True,
                    skip_group_check=True,
                )

            # --- all input DMAs ride ONE SP-ring queue in priority order:
            # tiny constants first (so per-row deps clear by ~9us), then the
            # whole hidden tensor.  The first two hidden chunks are 2 rows
            # each so row 0 can start as early as possible.  The SP engine
            # has no per-row duties, so trigger instructions blocking on
            # DMA-ring capacity cost nothing.
            hts = []  # (row_start, n_rows, tile)
            ht0 = hid_pool.tile([128, HDMA_GROUPS[0], KCH, T], FP8, tag="ht0", name="htt")
            nc.sync.dma_start(ht0[:], hid.ap()[:, 0 : HDMA_GROUPS[0]])
            hts.append((0, HDMA_GROUPS[0], ht0))
            wt = const_pool.tile([128, KCH, S], HP)
            nc.sync.dma_start(wt[:], w_d.ap()[:])
            # tiny per-row constants ride the otherwise-idle GPSIMD ring so
            # they don't queue behind hidden chunks
            segt = const_pool.tile([128, 2, rpc], F32)
            nc.gpsimd.dma_start(segt[:], seg_d.ap()[:])
            gst = const_pool.tile([128, 2, rpc], F32)
            nc.gpsimd.dma_start(gst[:], g_d.ap()[:])
            row0 = HDMA_GROUPS[0]
            for nrow in HDMA_GROUPS[1:]:
                ht = hid_pool.tile([128, nrow, KCH, T], FP8, tag=f"ht{nrow}", name="htt")
                nc.sync.dma_start(ht[:], hid.ap()[:, row0 : row0 + nrow])
                hts.append((row0, nrow, ht))
                row0 += nrow
            assert row0 == rpc

            def hid_slice(r):
                for row0, nrow, ht in hts:
                    if row0 <= r < row0 + nrow:
                        return ht[:, r - row0]
                raise AssertionError

            # dummy ACTIVATE pulls the lazy ACT table load into the idle
            # startup window instead of the first row's critical path
            wz2 = const_pool.tile([1, 8], HP)
            nc.scalar.copy(wz2[:], wz[0:1, 0:8])
            iota_i = const_pool.tile([128, T], mybir.dt.int32)
            nc.gpsimd.iota(iota_i[:], pattern=[[1, T]], base=0, channel_multiplier=0)
            iota16 = const_pool.tile([128, T], HP)
            nc.vector.tensor_copy(iota16[:], iota_i[:])
            if with_bias:
                ones = const_pool.tile([1, 128], HP)
                nc.vector.memset(ones[:], 1.0)
                bsb = const_pool.tile([1, S], HP)
                nc.scalar.dma_start(bsb[:], b_d.ap()[:])

            pending = []
            ob1s = {}

            def emit_stage2(item):
                # out[w, s] = sum_t Mg[t, w] * lsb[t, s]: Mg is the stationary
                # (two full M=128 w-chunks -- no 130-wide remainder group) and
                # lsb streams at N=130, halving stage-2 stream cycles vs the
                # lsb-stationary form.  Host transposes [w, s] -> [s, t].
                pr, plsb, pmg = item
                g8, rr8 = divmod(pr, 8)
                po = po1_pool.tile([128, 2, S], F32, tag="po")
                for wc in range(2):
                    for tch in range(2):
                        nc.tensor.matmul(
                            po[:, wc, :],
                            pmg[:, tch, 128 * wc : 128 * (wc + 1)],
                            plsb[:, tch, :],
                            start=(tch == 0),
                            stop=(tch == 1),
                            skip_group_check=True,
                        )
                if rr8 == 0:
                    ob1s[g8] = ob1_pool.tile([128, 8, 2, S], HP, tag="ob1", name="ob1t")
                ob1 = ob1s[g8]
                nc.scalar.copy(ob1[:, rr8, :, :], po[:])
                if g8 == 3 and rr8 == 3:
                    # split the last group's DMAs so the final transfer is small
                    nc.sync.dma_start(out_d.ap()[:, 24:28], ob1[:, 0:4])
                if g8 == 3 and rr8 == 5:
                    nc.sync.dma_start(out_d.ap()[:, 28:30], ob1[:, 4:6])
                if rr8 == 7:
                    if g8 == 3:
                        nc.sync.dma_start(out_d.ap()[:, 30:32], ob1[:, 6:8])
                    else:
                        nc.sync.dma_start(
                            out_d.ap()[:, 8 * g8 : 8 * (g8 + 1)], ob1[:]
                        )
                    del ob1s[g8]

            for r in range(rpc):
                ht_r = hid_slice(r)

                # Mg[t, w] = (seg[t] == w) * g[t]/s_t in fp16, one DVE
                # two-op tensor_scalar per t-chunk
                mg = mg_pool.tile([128, 2, T], HP, tag="mg")
                for c in range(2):
                    nc.vector.tensor_scalar(
                        mg[:, c, :],
                        iota16[:],
                        segt[:, c, r : r + 1],
                        gst[:, c, r : r + 1],
                        eq,
                        mul,
                    )

                # stage 1: scaled logits for both t-chunks into one PSUM bank
                psl = psl_pool.tile([128, 2, S], F32, tag="psl")
                for c in range(2):
                    for k in range(KCH):
                        nc.tensor.matmul(
                            psl[:, c, :],
                            ht_r[:, k, 128 * c : 128 * (c + 1)],
                            wt[:, k, :],
                            start=(k == 0),
                            stop=(k == KCH - 1 and not with_bias),
                            skip_group_check=True,
                        )
                    if with_bias:
                        nc.tensor.matmul(
                            psl[:, c, :], ones[:], bsb[:], start=False, stop=True,
                            skip_group_check=True,
                        )

                # PSUM -> SBUF pure cast, one ACT instruction per row
                lsb = lsb_pool.tile([128, 2, S], HP, tag="lsb")
                nc.scalar.activation(lsb[:], psl[:], copyf)

                # stage 2 is emitted one row late (software pipeline) so the
                # PE never waits on the ACT/DVE work of the same row.
                pending.append((r, lsb, mg))
                if len(pending) > 1:
                    emit_stage2(pending.pop(0))
            while pending:
                emit_stage2(pending.pop(0))

    if split_waits:
        _split_sync_waits(nc)
    return nc


def _host_prep(hidden, W, b, seg):
    """Layout/encoding prep: fp8 e3m4 per-token quantization of hidden,
    1/count of the integer segment ids, partition-major packing."""
    h = np.ascontiguousarray(hidden[:, 1:, :], dtype=np.float32)
    absmax = np.abs(h).max(axis=2, keepdims=True)  # [B, T, 1]
    s_t = (14.0 / np.maximum(absmax, 1e-30)).astype(np.float32)
    h8 = (h * s_t).astype(ml_dtypes.float8_e3m4)
    # [core][p, r, k, t] with p the SBUF partition (= h % 128 within chunk k)
    h8 = h8.reshape(N_CORES, RPC, T, KCH, 128)
    hiddenT = np.ascontiguousarray(h8.transpose(0, 4, 1, 3, 2))

    seg = np.asarray(seg)
    counts = np.zeros((B, T), dtype=np.int64)
    rows = np.arange(B)[:, None]
    np.add.at(counts, (rows, seg), 1)
    g = (1.0 / np.maximum(counts, 1))[rows, seg].astype(np.float32)  # [B, T]
    gs = (g / s_t[:, :, 0]).astype(np.float32)
    segf = seg.astype(np.float32)

    # partition-major packing: [core][p, c, r] = value at (row0+r, 128c+p)
    def pack(x):
        x4 = x.reshape(N_CORES, RPC, 2, 128)  # [core, r, c, p]
        return np.ascontiguousarray(x4.transpose(0, 3, 2, 1))

    segt = pack(segf)
    gst = pack(gs)
    w16 = np.asarray(W, dtype=np.float32).astype(np.float16).reshape(KCH, 128, S)
    w_in = np.ascontiguousarray(w16.transpose(1, 0, 2))  # [128, KCH, S]
    b_in = np.ascontiguousarray(b, dtype=np.float32).astype(np.float16).reshape(1, S)
    return hiddenT, w_in, b_in, segt, gst


_CACHE = {}


def kernel(hidden, W, b, seg):
    hiddenT, w_in, b_in, segt, gst = _host_prep(hidden, W, b, seg)
    with_bias = bool(np.any(b_in != 0.0))

    key = ("prog", with_bias)
    if key not in _CACHE:
        _CACHE[key] = _build_program(with_bias=with_bias)
    nc = _CACHE[key]

    in_maps = []
    for c in range(N_CORES):
        in_maps.append(
            {
                "hiddent": hiddenT[c],
                "w": w_in,
                "bvec": b_in,
                "segt": segt[c],
                "gst": gst[c],
            }
        )
    res = run_bass_kernel_spmd(nc, in_maps, core_ids=list(range(N_CORES)))
    # device layout: out [128(w%128), RPC, 2(w//128), S]; t = 128*wc + p
    out = np.empty((B, S, T), dtype=np.float32)
    for c in range(N_CORES):
        o = np.asarray(res.results[c]["out"], dtype=np.float32)
        out[c * RPC : (c + 1) * RPC] = (
            o.transpose(1, 3, 2, 0).reshape(RPC, S, T)
        )
    return out


# revision 39
# speedup vs baseline: 3.1095x; 1.0083x over previous
"""Trainium2 Bass kernel for nn_ModelIAS_53618371724066 (segment_reduce).

Computes, for each batch row b:
    logits = hidden[b, 1:, :] @ W + b_vec          # [T, S]
    merged[w, :] = mean over {t : seg[b,t] == w} of logits[t, :]   (0 if empty)
    out[b] = merged.T                               # [S, T]

Strategy (data-parallel over batch, 32 rows per core on 8 cores):
  - hidden is quantized host-side to fp8 e3m4 with a per-token scale s_t
    (absmax -> 14.0) so input DMA traffic halves vs fp16; W stays fp16
    (its small magnitudes fall into the e3m4 denormal range).  The matmul
    runs mixed fp8 x fp16 at the bf16 stream rate with fp8 FWL weight
    loads.  Measured end-to-end max rel err ~1.4e-2 vs the 2e-2 gate.
  - The scatter matrix Mg[t, w] = (seg[t] == w) * g[t]/s_t is built in
    ONE DVE tensor_scalar per t-chunk using the two-op form
    (is_equal then mult) -- folding the mean weight g and the fp8
    de-scale into Mg.  This makes the PSUM->SBUF logits copy a pure
    cast, done as a single ACT instruction per row over [128, 2, S].
  - Stage 1 (PE): logits[t_chunk, s] = sum_k hidT[k-chunk].T @ W[k-chunk]
    accumulated in one fp32 PSUM bank per row ([128, 2, S]).
  - Stage 2 (PE): out[s, w] = sum_c lsb_c.T @ Mg_c.  s-channels 0:128 go
    to po1 [128, T]; the 130-wide remainder (2 channels) accumulates into
    a shared [2, 2, T] bank per row-pair so its PSUM->SBUF copy and DMA
    amortize over 2 rows.
  - Outputs leave the chip in fp16 (host converts to f32): po1 is cast by
    DVE into a [128, 8, T] staging tile (one DMA per 8 rows, the last
    group split small so the end-of-program barrier isn't held by a big
    transfer), po2 by ACT into a [128, 4, T] tile.
  - A ~3.5us burst of 8 wide dummy matmuls at program start warms the PE
    HAM clock gate (1.2 -> 2.4 GHz); it is sized to end right as the
    first hidden chunk lands so the PE busy-window never breaks (any
    idle gap resets the HAM un-throttle progress).
  - DMA plumbing: one SP-ring queue carries, in strict FIFO priority,
    a small 2-row first hidden chunk, W, then the remaining hidden
    (whole input fits SBUF) and all output DMAs; the tiny segt/gst
    constants ride the otherwise-idle GPSIMD ring.  DMA trigger
    instructions cost ~0.6us on the issuing engine and block on ring
    capacity, so none may live on ACT/DVE which do per-row work.
  - A dummy ACTIVATE after program start pulls the lazy 1.3us ACT table
    load into the startup window instead of row 0's critical path.
  - Per-instruction sem-waits are legalized for the pinned walrus by
    _split_sync_waits.

Measured 53.4us (from 73.9us baseline): preamble ~8 | warmup ~3.5 |
32 rows x 1134ns | tail+teardown ~4.  The row period is the PE floor:
stage1 streams at the 54ns/matmul fp16 rate and stage2 at 108ns/matmul
(and the DVE is co-bound at ~1095ns/row of Mg-build + output-cast work).
"""

import numpy as np
import ml_dtypes

import concourse.bass as bass
import concourse.tile as tile
from concourse import mybir
from concourse.bass_utils import run_bass_kernel_spmd

B, T, H, S = 256, 256, 768, 130
N_CORES = 8
RPC = B // N_CORES  # rows per core
KCH = H // 128  # k chunks of the hidden dim
F32 = mybir.dt.float32
HP = mybir.dt.float16
FP8 = mybir.dt.float8e3  # e3m4
N_WARM = 7
ROWS_PER_HDMA = 4
HDMA_GROUPS = [2, 2, 4, 4, 4, 4, 4, 4, 4]


def _split_sync_waits(nc):
    """The pinned walrus build rejects instructions carrying more than one
    sync-wait command ("Too many sync wait commands", setupSyncWait).  Keep
    one wait per instruction and hoist the rest onto NoOps inserted just
    before it on the same engine (same semantics: all waits still execute
    before the instruction, in stream order)."""
    for f in nc.m.functions:
        for blk in f.blocks:
            il = blk.instructions
            i = 0
            while i < len(il):
                inst = il[i]
                si = inst.sync_info
                if si is not None and si.on_wait and len(si.on_wait) >= 2:
                    waits = list(si.on_wait)
                    keep = [waits.pop()]
                    pos = i
                    for j, w in enumerate(waits):
                        nop = mybir.InstNoOp(name=f"{inst.name}_ws{j}", ins=[], outs=[])
                        nop.engine = inst.engine
                        nop.sync_info = mybir.SyncInfo(on_wait=[w], on_update=[])
                        il.insert(pos, nop)
                        pos += 1
                        i += 1
                    inst.sync_info = mybir.SyncInfo(
                        on_wait=keep, on_update=list(si.on_update)
                    )
                i += 1


def _build_program(rpc=RPC, with_bias=False, split_waits=True):
    nc = bass.Bass("TRN2", target_bir_lowering=False, debug=False)

    hid = nc.dram_tensor("hiddent", [128, rpc, KCH, T], FP8, kind="ExternalInput")
    w_d = nc.dram_tensor("w", [128, KCH, S], HP, kind="ExternalInput")
    b_d = nc.dram_tensor("bvec", [1, S], HP, kind="ExternalInput")
    seg_d = nc.dram_tensor("segt", [128, 2, rpc], F32, kind="ExternalInput")
    g_d = nc.dram_tensor("gst", [128, 2, rpc], F32, kind="ExternalInput")
    out_d = nc.dram_tensor("out", [128, rpc, 2, S], HP, kind="ExternalOutput")

    eq = mybir.AluOpType.is_equal
    mul = mybir.AluOpType.mult
    copyf = mybir.ActivationFunctionType.Copy
    assert rpc % ROWS_PER_HDMA == 0
    ngrp = rpc // ROWS_PER_HDMA
    with tile.TileContext(nc) as tc:
        with (
            tc.tile_pool(name="const", bufs=1) as const_pool,
            tc.tile_pool(name="hid", bufs=len(HDMA_GROUPS)) as hid_pool,
            tc.tile_pool(name="mgp", bufs=4) as mg_pool,
            tc.tile_pool(name="lsbp", bufs=4) as lsb_pool,
            tc.tile_pool(name="ob1p", bufs=4) as ob1_pool,
            tc.tile_pool(name="warm", bufs=1, space=bass.MemorySpace.PSUM) as warm_pool,
            tc.tile_pool(name="psl", bufs=3, space=bass.MemorySpace.PSUM) as psl_pool,
            tc.tile_pool(name="po1", bufs=3, space=bass.MemorySpace.PSUM) as po1_pool,
        ):
            # --- HAM warmup: keep the PE busy from t~0 so the clock gate
            # opens (K=8/8) before real matmuls arrive.  Dummy data.
            wz = const_pool.tile([128, 512], HP)
            nc.vector.memset(wz[:], 0.0)
            pwu = warm_pool.tile([64, 512], F32)
            for i in range(N_WARM):
                nc.tensor.matmul(
                    pwu[:, :],
                    wz[:, 0:64],
                    wz[:, :],
                    start=True,
                    stop=True,
                    skip_group_check=True,
                )

            # --- all input DMAs ride ONE SP-ring queue in priority order:
            # tiny constants first (so per-row deps clear by ~9us), then the
            # whole hidden tensor.  The first two hidden chunks are 2 rows
            # each so row 0 can start as early as possible.  The SP engine
            # has no per-row duties, so trigger instructions blocking on
            # DMA-ring capacity cost nothing.
            hts = []  # (row_start, n_rows, tile)
            ht0 = hid_pool.tile([128, HDMA_GROUPS[0], KCH, T], FP8, tag="ht0", name="htt")
            nc.sync.dma_start(ht0[:], hid.ap()[:, 0 : HDMA_GROUPS[0]])
            hts.append((0, HDMA_GROUPS[0], ht0))
            wt = const_pool.tile([128, KCH, S], HP)
            nc.sync.dma_start(wt[:], w_d.ap()[:])
            # tiny per-row constants ride the otherwise-idle GPSIMD ring so
            # they don't queue behind hidden chunks
            segt = const_pool.tile([128, 2, rpc], F32)
            nc.gpsimd.dma_start(segt[:], seg_d.ap()[:])
            gst = const_pool.tile([128, 2, rpc], F32)
            nc.gpsimd.dma_start(gst[:], g_d.ap()[:])
            row0 = HDMA_GROUPS[0]
            for nrow in HDMA_GROUPS[1:]:
                ht = hid_pool.tile([128, nrow, KCH, T], FP8, tag=f"ht{nrow}", name="htt")
                nc.sync.dma_start(ht[:], hid.ap()[:, row0 : row0 + nrow])
                hts.append((row0, nrow, ht))
                row0 += nrow
            assert row0 == rpc

            def hid_slice(r):
                for row0, nrow, ht in hts:
                    if row0 <= r < row0 + nrow:
                        return ht[:, r - row0]
                raise AssertionError

            # dummy ACTIVATE pulls the lazy ACT table load into the idle
            # startup window instead of the first row's critical path
            wz2 = const_pool.tile([1, 8], HP)
            nc.scalar.copy(wz2[:], wz[0:1, 0:8])
            iota_i = const_pool.tile([128, T], mybir.dt.int32)
            nc.gpsimd.iota(iota_i[:], pattern=[[1, T]], base=0, channel_multiplier=0)
            iota16 = const_pool.tile([128, T], HP)
            nc.vector.tensor_copy(iota16[:], iota_i[:])
            if with_bias:
                ones = const_pool.tile([1, 128], HP)
                nc.vector.memset(ones[:], 1.0)
                bsb = const_pool.tile([1, S], HP)
                nc.scalar.dma_start(bsb[:], b_d.ap()[:])

            pending = []
            ob1s = {}

            def emit_stage2(item):
                # out[w, s] = sum_t Mg[t, w] * lsb[t, s]: Mg is the stationary
                # (two full M=128 w-chunks -- no 130-wide remainder group) and
                # lsb streams at N=130, halving stage-2 stream cycles vs the
                # lsb-stationary form.  Host transposes [w, s] -> [s, t].
                pr, plsb, pmg = item
                g8, rr8 = divmod(pr, 8)
                po = po1_pool.tile([128, 2, S], F32, tag="po")
                for wc in range(2):
                    for tch in range(2):
                        nc.tensor.matmul(
                            po[:, wc, :],
                            pmg[:, tch, 128 * wc : 128 * (wc + 1)],
                            plsb[:, tch, :],
                            start=(tch == 0),
                            stop=(tch == 1),
                            skip_group_check=True,
                        )
                if rr8 == 0:
                    ob1s[g8] = ob1_pool.tile([128, 8, 2, S], HP, tag="ob1", name="ob1t")
                ob1 = ob1s[g8]
                nc.scalar.copy(ob1[:, rr8, :, :], po[:])
                if g8 == 3 and rr8 == 3:
                    # split the last group's DMAs so the final transfer is small
                    nc.sync.dma_start(out_d.ap()[:, 24:28], ob1[:, 0:4])
                if g8 == 3 and rr8 == 5:
                    nc.sync.dma_start(out_d.ap()[:, 28:30], ob1[:, 4:6])
                if rr8 == 7:
                    if g8 == 3:
                        nc.sync.dma_start(out_d.ap()[:, 30:32], ob1[:, 6:8])
                    else:
                        nc.sync.dma_start(
                            out_d.ap()[:, 8 * g8 : 8 * (g8 + 1)], ob1[:]
                        )
                    del ob1s[g8]

            for r in range(rpc):
                ht_r = hid_slice(r)

                # Mg[t, w] = (seg[t] == w) * g[t]/s_t in fp16, one DVE
                # two-op tensor_scalar per t-chunk
                mg = mg_pool.tile([128, 2, T], HP, tag="mg")
                for c in range(2):
                    nc.vector.tensor_scalar(
                        mg[:, c, :],
                        iota16[:],
                        segt[:, c, r : r + 1],
                        gst[:, c, r : r + 1],
                        eq,
                        mul,
                    )

                # stage 1: scaled logits for both t-chunks into one PSUM bank
                psl = psl_pool.tile([128, 2, S], F32, tag="psl")
                for c in range(2):
                    for k in range(KCH):
                        nc.tensor.matmul(
                            psl[:, c, :],
                            ht_r[:, k, 128 * c : 128 * (c + 1)],
                            wt[:, k, :],
                            start=(k == 0),
                            stop=(k == KCH - 1 and not with_bias),
                            skip_group_check=True,
                        )
                    if with_bias:
                        nc.tensor.matmul(
                            psl[:, c, :], ones[:], bsb[:], start=False, stop=True,
                            skip_group_check=True,
                        )

                # PSUM -> SBUF pure cast, one ACT instruction per row
                lsb = lsb_pool.tile([128, 2, S], HP, tag="lsb")
                nc.scalar.activation(lsb[:], psl[:], copyf)

                # stage 2 is emitted one row late (software pipeline) so the
                # PE never waits on the ACT/DVE work of the same row.
                pending.append((r, lsb, mg))
                if len(pending) > 1:
                    emit_stage2(pending.pop(0))
            while pending:
                emit_stage2(pending.pop(0))

    if split_waits:
        _split_sync_waits(nc)
    return nc


def _host_prep(hidden, W, b, seg):
    """Layout/encoding prep: fp8 e3m4 per-token quantization of hidden,
    1/count of the integer segment ids, partition-major packing."""
    h = np.ascontiguousarray(hidden[:, 1:, :], dtype=np.float32)
    absmax = np.abs(h).max(axis=2, keepdims=True)  # [B, T, 1]
    s_t = (14.0 / np.maximum(absmax, 1e-30)).astype(np.float32)
    h8 = (h * s_t).astype(ml_dtypes.float8_e3m4)
    # [core][p, r, k, t] with p the SBUF partition (= h % 128 within chunk k)
    h8 = h8.reshape(N_CORES, RPC, T, KCH, 128)
    hiddenT = np.ascontiguousarray(h8.transpose(0, 4, 1, 3, 2))

    seg = np.asarray(seg)
    counts = np.zeros((B, T), dtype=np.int64)
    rows = np.arange(B)[:, None]
    np.add.at(counts, (rows, seg), 1)
    g = (1.0 / np.maximum(counts, 1))[rows, seg].astype(np.float32)  # [B, T]
    gs = (g / s_t[:, :, 0]).astype(np.float32)
    segf = seg.astype(np.float32)

    # partition-major packing: [core][p, c, r] = value at (row0+r, 128c+p)
    def pack(x):
        x4 = x.reshape(N_CORES, RPC, 2, 128)  # [core, r, c, p]
        return np.ascontiguousarray(x4.transpose(0, 3, 2, 1))

    segt = pack(segf)
    gst = pack(gs)
    w16 = np.asarray(W, dtype=np.float32).astype(np.float16).reshape(KCH, 128, S)
    w_in = np.ascontiguousarray(w16.transpose(1, 0, 2))  # [128, KCH, S]
    b_in = np.ascontiguousarray(b, dtype=np.float32).astype(np.float16).reshape(1, S)
    return hiddenT, w_in, b_in, segt, gst


_CACHE = {}


def kernel(hidden, W, b, seg):
    hiddenT, w_in, b_in, segt, gst = _host_prep(hidden, W, b, seg)
    with_bias = bool(np.any(b_in != 0.0))

    key = ("prog", with_bias)
    if key not in _CACHE:
        _CACHE[key] = _build_program(with_bias=with_bias)
    nc = _CACHE[key]

    in_maps = []
    for c in range(N_CORES):
        in_maps.append(
            {
                "hiddent": hiddenT[c],
                "w": w_in,
                "bvec": b_in,
                "segt": segt[c],
                "gst": gst[c],
            }
        )
    res = run_bass_kernel_spmd(nc, in_maps, core_ids=list(range(N_CORES)))
    # device layout: out [128(w%128), RPC, 2(w//128), S]; t = 128*wc + p
    out = np.empty((B, S, T), dtype=np.float32)
    for c in range(N_CORES):
        o = np.asarray(res.results[c]["out"], dtype=np.float32)
        out[c * RPC : (c + 1) * RPC] = (
            o.transpose(1, 3, 2, 0).reshape(RPC, S, T)
        )
    return out
